# revision 19
# baseline (speedup 1.0000x reference)
"""Distributed GQA attention (B=2, S=2048, H=2048, 32 heads / 8 KV heads,
RoPE, causal) on 8 TRN2 NeuronCores.

Sharding: core c -> (batch b = c//4, head-group hg = c%4).
Each core computes q-heads [8hg, 8hg+8) and kv-heads [2hg, 2hg+2) of its
batch, runs attention locally (GQA groups stay on-core), then the four
cores of a batch AllGather their attention outputs (bf16) and each
computes a disjoint 512-column slice of the output projection, so no
all-reduce is needed.  Host reassembles the 8 disjoint slices.

Device layouts are transposed ([channel, row]) so RoPE / QK / O-proj
contract along partitions; softmax runs without max-subtraction
(scores are bounded) and denominators come from a ones-column appended
to V.  AV is computed with exp-scores as the stationary operand so the
output lands [q-partitions, head-dim] and the softmax denominator is a
per-partition column (cheap reciprocal + per-partition scale), then
PE-transposed back to [channel, q].  The V bias is folded into the
output bias on host (softmax rows sum to 1).  Attention is split into
7 column phases; each phase's AllGather overlaps later attention and
its out-proj slice is deferred three phases (absorbing cross-core
launch skew at the first collective) and interleaved into later
attention as PE filler, as are the V-projection tiles.
"""
import os
import sys

sys.path.insert(0, "/opt/trn_rl_repo")

import numpy as np
import ml_dtypes

import concourse.bass as bass
import concourse.mybir as mybir
import concourse.tile as tile
from concourse import bacc
from concourse import bass_utils
from concourse.masks import make_identity

BF16 = mybir.dt.bfloat16
F32 = mybir.dt.float32
ADD = mybir.AluOpType.add
MULT = mybir.AluOpType.mult

B, S, H = 2, 2048, 2048
NH, NKV, HD = 32, 8, 64
SCALE = HD ** -0.5
RG = [[0, 1, 2, 3], [4, 5, 6, 7]]
N_CORES = 8
NT = S // 128          # 16 seq tiles
HT = H // 128          # 16 hidden tiles

# attention phases: qt ranges (balanced by causal work)
PH = [(0, 4), (4, 8), (8, 11), (11, 13), (13, 14), (14, 15), (15, 16)]
PC0 = [128 * lo for lo, _ in PH]           # column offset per phase
PW = [128 * (hi - lo) for lo, hi in PH]    # column width per phase

TRACE = os.environ.get("BASS_KERNEL_TRACE", "0") == "1"
LAST_EXEC_NS = None
_COMPILED = None


def _install_profile_shim():
    import types
    try:
        from trn_agent_boot.trn_boot import _ntff_profile_via_ctypes
    except ImportError:
        return
    hook = _ntff_profile_via_ctypes("/opt/axon/libaxon_pjrt.so")
    mod = types.ModuleType("antenv.axon_hooks")
    mod.get_axon_ntff_profile_hook = lambda: hook
    mod.set_axon_ntff_profile_hook = lambda h: None
    sys.modules["antenv.axon_hooks"] = mod
    bass_utils.upload_artifacts = lambda tmpdir: tmpdir


def _build():
    nc = bacc.Bacc("TRN2", target_bir_lowering=False, debug=False,
                   num_devices=N_CORES)

    xt = nc.dram_tensor("xt", [H, S], BF16, kind="ExternalInput")
    wqt = nc.dram_tensor("wqt", [128, HT * 512], BF16, kind="ExternalInput")
    wkt = nc.dram_tensor("wkt", [128, HT * 128], BF16, kind="ExternalInput")
    wvt = nc.dram_tensor("wvt", [128, HT * 128], BF16, kind="ExternalInput")
    wot = nc.dram_tensor("wot", [128, HT * 512], BF16, kind="ExternalInput")
    bq = nc.dram_tensor("bq", [512, 1], F32, kind="ExternalInput")
    bk = nc.dram_tensor("bk", [128, 1], F32, kind="ExternalInput")
    bo = nc.dram_tensor("bo", [512, 1], F32, kind="ExternalInput")
    kcos = nc.dram_tensor("kcos", [128, S], BF16, kind="ExternalInput")
    ksin = nc.dram_tensor("ksin", [128, S], BF16, kind="ExternalInput")
    maskd = nc.dram_tensor("maskd", [128, 128], F32, kind="ExternalInput")
    out = nc.dram_tensor("out", [512, S], F32, kind="ExternalOutput")

    Exp = mybir.ActivationFunctionType.Exp

    from contextlib import ExitStack
    with tile.TileContext(nc) as tc:
        with ExitStack() as stk:
            ep = stk.enter_context
            big = ep(tc.tile_pool(name="big", bufs=16))     # xt / gathered o
            wpool = ep(tc.tile_pool(name="w", bufs=2))      # wqt / wot
            wkpool = ep(tc.tile_pool(name="wk", bufs=1))
            wvpool = ep(tc.tile_pool(name="wv", bufs=1))
            qpool = ep(tc.tile_pool(name="qt", bufs=4))
            kpool = ep(tc.tile_pool(name="kt", bufs=2))
            vpool = ep(tc.tile_pool(name="vv", bufs=16))
            opool = ep(tc.tile_pool(name="ot", bufs=4))
            tabpool = ep(tc.tile_pool(name="tab", bufs=4))
            mkpool = ep(tc.tile_pool(name="mk", bufs=1))
            ropepool = ep(tc.tile_pool(name="rope", bufs=6))
            expool = ep(tc.tile_pool(name="exp", bufs=6))
            nrmpool = ep(tc.tile_pool(name="nrm", bufs=4))
            ypool = ep(tc.tile_pool(name="yy", bufs=2))
            bpool = ep(tc.tile_pool(name="bias", bufs=12))
            idpool = ep(tc.tile_pool(name="id", bufs=1))
            pp = ep(tc.tile_pool(name="pp", bufs=1, space="PSUM"))
            scp = ep(tc.tile_pool(name="sc", bufs=2, space="PSUM"))
            avp = ep(tc.tile_pool(name="av", bufs=2, space="PSUM"))
            tpp = ep(tc.tile_pool(name="tp", bufs=1, space="PSUM"))
            dram = ep(tc.tile_pool(name="dram", bufs=1, space="DRAM"))

            # ---------- input loads: small weights first, then xt ----------
            wk_sb = wkpool.tile([128, HT * 128], BF16, name="wk", tag="wk")
            nc.sync.dma_start(out=wk_sb[:, 0:1024], in_=wkt[:, 0:1024])
            nc.sync.dma_start(out=wk_sb[:, 1024:2048], in_=wkt[:, 1024:2048])
            wv_sb = wvpool.tile([128, HT * 128], BF16, name="wv", tag="wv")
            nc.sync.dma_start(out=wv_sb[:, 0:1024], in_=wvt[:, 0:1024])
            nc.sync.dma_start(out=wv_sb[:, 1024:2048], in_=wvt[:, 1024:2048])
            bq_sb, bo_sb = [], []
            for o in range(4):
                b_t = bpool.tile([128, 1], F32, name=f"bq{o}", tag="bias")
                nc.sync.dma_start(out=b_t[:, :], in_=bq[128 * o:128 * (o + 1), :])
                bq_sb.append(b_t)
            bk_sb = bpool.tile([128, 1], F32, name="bk", tag="bias")
            nc.sync.dma_start(out=bk_sb[:, :], in_=bk[:, :])
            for o in range(4):
                b_t = bpool.tile([128, 1], F32, name=f"bo{o}", tag="bias")
                nc.sync.dma_start(out=b_t[:, :], in_=bo[128 * o:128 * (o + 1), :])
                bo_sb.append(b_t)
            ident = idpool.tile([128, 128], BF16, name="ident", tag="id")
            make_identity(nc, ident[:, :])
            xt_sb = []
            for t in range(HT):
                x_t = big.tile([128, S], BF16, name=f"xt{t}", tag="big")
                nc.sync.dma_start(out=x_t[:, 0:1024],
                                  in_=xt[128 * t:128 * (t + 1), 0:1024])
                nc.sync.dma_start(out=x_t[:, 1024:2048],
                                  in_=xt[128 * t:128 * (t + 1), 1024:2048])
                xt_sb.append(x_t)
            wq_sb = wpool.tile([128, HT * 512], BF16, name="wq", tag="w")
            nc.sync.dma_start(out=wq_sb[:, :], in_=wqt[:, :])
            kcos_sb = tabpool.tile([128, S], BF16, name="kcos", tag="tab")
            nc.sync.dma_start(out=kcos_sb[:, :], in_=kcos[:, :])
            ksin_sb = tabpool.tile([128, S], BF16, name="ksin", tag="tab")
            nc.sync.dma_start(out=ksin_sb[:, :], in_=ksin[:, :])
            maskd_sb = mkpool.tile([128, 128], F32, name="maskd", tag="mk")
            nc.sync.dma_start(out=maskd_sb[:, :], in_=maskd[:, :])

            def rope(psum, bias_ap, cos_sb, sin_sb, c, out_ap):
                """out = (psum+bias)*cos + shift32((psum+bias)*sin_pre).

                The psum+bias runs on the scalar engine so the projection
                PSUM slot is released after one short op."""
                cs = slice(512 * c, 512 * (c + 1))
                tb = ropepool.tile([128, 512], F32, name="tb", tag="rope")
                nc.scalar.activation(tb[:, :], psum[:, :],
                                     mybir.ActivationFunctionType.Identity,
                                     bias=bias_ap)
                tcos = ropepool.tile([128, 512], F32, name="tcos", tag="rope")
                nc.vector.tensor_tensor(tcos[:, :], tb[:, :], cos_sb[:, cs],
                                        MULT)
                tsin = ropepool.tile([128, 512], F32, name="tsin", tag="rope")
                nc.vector.tensor_tensor(tsin[:, :], tb[:, :], sin_sb[:, cs],
                                        MULT)
                tsh = ropepool.tile([128, 512], F32, name="tsh", tag="rope")
                for d, s in ((0, 32), (32, 0), (64, 96), (96, 64)):
                    nc.sync.dma_start(out=tsh[d:d + 32, :], in_=tsin[s:s + 32, :])
                nc.vector.tensor_tensor(out_ap, tcos[:, :], tsh[:, :], ADD)

            # ---------- K projection + rope ----------
            kT_sb = kpool.tile([128, S], BF16, name="kT", tag="kt")
            kT_sw = kpool.tile([128, S], BF16, name="kTswap", tag="kt")
            for c in range(4):
                ps = pp.tile([128, 512], F32, name="psk", tag="pp")
                for t in range(HT):
                    nc.tensor.matmul(ps[:, :],
                                     wk_sb[:, 128 * t:128 * (t + 1)],
                                     xt_sb[t][:, 512 * c:512 * (c + 1)],
                                     start=(t == 0), stop=(t == HT - 1))
                rope(ps, bk_sb[:, :], kcos_sb, ksin_sb, c,
                     kT_sb[:, 512 * c:512 * (c + 1)])
            # kT_sw: swapped kv halves (kv1 on partitions 0:64, kv0 on 64:128)
            nc.sync.dma_start(out=kT_sw[0:64, :], in_=kT_sb[64:128, :])
            nc.sync.dma_start(out=kT_sw[64:128, :], in_=kT_sb[0:64, :])

            # ---------- V projection (layout [rows, oc], 65-strided + ones) ----------
            # emitted lazily: early tiles up front, the rest as filler
            # between attention phases (v[rt] is first read at qt == rt)
            v_sb = [vpool.tile([128, 130], BF16, name=f"v{rt}", tag="v")
                    for rt in range(NT)]

            def emit_vproj(rt):
                v_t = v_sb[rt]
                nc.gpsimd.memset(
                    v_t[:, :].rearrange("p (m c) -> p m c", c=65)[:, :, 64:65], 1.0)
                ps = pp.tile([128, 128], F32, name="psv", tag="pp")
                for t in range(HT):
                    nc.tensor.matmul(ps[:, :],
                                     xt_sb[t][:, 128 * rt:128 * (rt + 1)],
                                     wv_sb[:, 128 * t:128 * (t + 1)],
                                     start=(t == 0), stop=(t == HT - 1))
                nc.scalar.activation(
                    v_t[:, :].rearrange("p (m c) -> p m c", c=65)[:, :, 0:64],
                    ps[:, :].rearrange("p (m c) -> p m c", c=64),
                    mybir.ActivationFunctionType.Copy)

            for rt in range(4):
                emit_vproj(rt)

            # ---------- Q projection + rope (scale folded into tables) ----------
            qT_sb = [None] * 4

            def emit_qproj(o):
                q_t = qpool.tile([128, S], BF16, name=f"qT{o}", tag="qt")
                qT_sb[o] = q_t
                for c in range(4):
                    ps = pp.tile([128, 512], F32, name="psq", tag="pp")
                    for t in range(HT):
                        nc.tensor.matmul(ps[:, :],
                                         wq_sb[:, 512 * t + 128 * o:
                                               512 * t + 128 * (o + 1)],
                                         xt_sb[t][:, 512 * c:512 * (c + 1)],
                                         start=(t == 0), stop=(t == HT - 1))
                    rope(ps, bq_sb[o][:, :], kcos_sb, ksin_sb, c,
                         q_t[:, 512 * c:512 * (c + 1)])

            # ---------- attention ----------
            oT_sb = []
            for j in range(4):
                o_t = opool.tile([128, S], BF16, name=f"oT{j}", tag="ot")
                oT_sb.append(o_t)

            NPH = len(PH)
            bounce = [dram.tile([512, PW[p]], BF16, name=f"bounce{p}",
                                tag=f"bounce{p}") for p in range(NPH)]
            gath = [dram.tile([4, 512, PW[p]], BF16, name=f"gath{p}",
                              tag=f"gath{p}") for p in range(NPH)]
            gsb_all = [None] * NPH
            wo_sb = [None]

            def emit_attn_chain(j, qt):
                """One (head-pair, q-tile) attention chain; chains from
                different j are independent and pipeline through PE/scalar."""
                kvl = j // 2
                ke = kT_sb if kvl == 0 else kT_sw
                ko = kT_sw if kvl == 0 else kT_sb
                qs = slice(128 * qt, 128 * (qt + 1))
                av_e = avp.tile([128, 65], F32, name="ave", tag="av")
                av_o = avp.tile([128, 65], F32, name="avo", tag="av")

                def emit_scores(k0, width):
                    sc2 = scp.tile([128, 1024], F32, name="sc2", tag="sc")
                    for cc in range(width):
                        kt = k0 + cc
                        cs = slice(128 * cc, 128 * (cc + 1))
                        co = slice(512 + 128 * cc, 512 + 128 * (cc + 1))
                        ks = slice(128 * kt, 128 * (kt + 1))
                        nc.tensor.matmul(sc2[:, cs], ke[0:64, ks],
                                         qT_sb[j][0:64, qs],
                                         start=True, stop=True,
                                         tile_position=(0, 0))
                        nc.tensor.matmul(sc2[:, co], ko[64:128, ks],
                                         qT_sb[j][64:128, qs],
                                         start=True, stop=True,
                                         tile_position=(64, 0))
                        if kt == qt:
                            nc.vector.tensor_tensor(sc2[:, cs], sc2[:, cs],
                                                    maskd_sb[:, :], ADD)
                            nc.vector.tensor_tensor(sc2[:, co], sc2[:, co],
                                                    maskd_sb[:, :], ADD)
                    return sc2

                def emit_expav(sc2, k0, width):
                    w = 128 * width
                    eb2 = expool.tile([128, 1024], BF16, name="eb2", tag="exp")
                    nc.scalar.activation(eb2[:, 0:w], sc2[:, 0:w], Exp)
                    nc.scalar.activation(eb2[:, 512:512 + w],
                                         sc2[:, 512:512 + w], Exp)
                    for par, base in ((0, 0), (1, 512)):
                        avt = (av_e if par == 0 else av_o)[:, :]
                        for cc in range(width):
                            ktt = k0 + cc
                            vs = v_sb[ktt][:, 65 * kvl:65 * kvl + 65]
                            ccs = slice(base + 128 * cc, base + 128 * (cc + 1))
                            nc.tensor.matmul(avt, eb2[:, ccs], vs,
                                             start=(ktt == 0),
                                             stop=(ktt == qt))

                chunks = [(k0, min(4, qt + 1 - k0))
                          for k0 in range(0, qt + 1, 4)]
                prev = None
                for ch in chunks:
                    sc2 = emit_scores(*ch)
                    if prev is not None:
                        emit_expav(*prev)
                    prev = (sc2, *ch)
                emit_expav(*prev)
                # normalize ([q, d] layout: denom is a per-partition col),
                # then PE-transpose back to [d, q]
                ps_t = tpp.tile([128, 128], BF16, name="pst", tag="tp")
                for par, av in ((0, av_e), (1, av_o)):
                    rc = nrmpool.tile([128, 1], F32, name="rc", tag="rc")
                    nc.vector.reciprocal(rc[:, :], av[:, 64:65])
                    nm = nrmpool.tile([128, 64], BF16, name="nm", tag="nm")
                    nc.vector.tensor_scalar_mul(nm[:, :], av[:, 0:64],
                                                rc[:, :])
                    nc.tensor.transpose(ps_t[64 * par:64 * (par + 1), :],
                                        nm[:, :], ident[:, :])
                nc.vector.tensor_copy(oT_sb[j][:, qs], ps_t[:, :])

            def emit_gather(p):
                for jj in range(4):
                    nc.sync.dma_start(
                        out=bounce[p][128 * jj:128 * (jj + 1), :],
                        in_=oT_sb[jj][:, PC0[p]:PC0[p] + PW[p]])
                nc.gpsimd.collective_compute(
                    "AllGather", mybir.AluOpType.bypass, replica_groups=RG,
                    ins=[bounce[p][:, :].opt()],
                    outs=[gath[p][:, :, :].opt()])
                # prefetch the gathered tiles while later attention runs
                W = PW[p]
                gview = gath[p][:, :, :].rearrange("g i q -> (g i) q")
                gsb = []
                for t in range(HT):
                    g_t = big.tile([128, W], BF16, name=f"g{p}_{t}", tag="big")
                    nc.sync.dma_start(out=g_t[:, :],
                                      in_=gview[128 * t:128 * (t + 1), :])
                    gsb.append(g_t)
                gsb_all[p] = gsb

            def emit_wo_loads():
                w_t = wpool.tile([128, HT * 512], BF16, name="wo", tag="w")
                nc.sync.dma_start(out=w_t[:, :], in_=wot[:, :])
                wo_sb[0] = w_t

            def oproj_chunks(p):
                W = PW[p]
                gsb = gsb_all[p]
                for o in range(4):
                    for q0 in range(0, W, 512):
                        yield (p, o, q0, min(512, W - q0), gsb)

            def emit_oproj_chunk(chunk):
                p, o, q0, cw, gsb = chunk
                ps = pp.tile([128, cw], F32, name="psy", tag="pp")
                for t in range(HT):
                    nc.tensor.matmul(
                        ps[:, :],
                        wo_sb[0][:, 512 * t + 128 * o:
                                 512 * t + 128 * (o + 1)],
                        gsb[t][:, q0:q0 + cw],
                        start=(t == 0), stop=(t == HT - 1))
                y_t = ypool.tile([128, cw], F32, name="y", tag="y")
                nc.vector.tensor_scalar_add(y_t[:, :], ps[:, :],
                                            bo_sb[o][:, :])
                nc.sync.dma_start(
                    out=out[128 * o:128 * (o + 1),
                            PC0[p] + q0:PC0[p] + q0 + cw],
                    in_=y_t[:, :])

            def emit_oproj(p):
                for ch in oproj_chunks(p):
                    emit_oproj_chunk(ch)

            # phase 0 interleaved with Q projection; late V tiles as filler
            for j in range(4):
                emit_qproj(j)
                for qt in range(*PH[0]):
                    emit_attn_chain(j, qt)
                emit_vproj(4 + j)
            emit_wo_loads()
            emit_gather(0)
            for j in range(4):
                for qt in range(*PH[1]):
                    emit_attn_chain(j, qt)
                if 8 + j < NT:
                    emit_vproj(8 + j)
            emit_gather(1)
            for j in range(4):
                for qt in range(*PH[2]):
                    emit_attn_chain(j, qt)
                if 12 + j < NT:
                    emit_vproj(12 + j)
            emit_gather(2)
            for p in range(3, NPH):
                pending = list(oproj_chunks(p - 3))
                nchunk = len(pending)
                for j in range(4):
                    for qt in range(*PH[p]):
                        emit_attn_chain(j, qt)
                    for ch in pending[(nchunk * j + 3) // 4:
                                      (nchunk * (j + 1) + 3) // 4]:
                        emit_oproj_chunk(ch)
                emit_gather(p)
            emit_oproj(NPH - 3)
            emit_oproj(NPH - 2)
            emit_oproj(NPH - 1)

    nc.compile()
    return nc


def kernel(**inputs):
    global _COMPILED, LAST_EXEC_NS
    x = np.asarray(inputs["hidden_states"], dtype=np.float32)
    mask = np.asarray(inputs["attention_mask"], dtype=np.float32)
    pos = np.asarray(inputs["position_ids"])
    Wq = np.asarray(inputs["Wq"], dtype=np.float32)
    bq = np.asarray(inputs["bq"], dtype=np.float32)
    Wk = np.asarray(inputs["Wk"], dtype=np.float32)
    bk = np.asarray(inputs["bk"], dtype=np.float32)
    Wv = np.asarray(inputs["Wv"], dtype=np.float32)
    bv = np.asarray(inputs["bv"], dtype=np.float32)
    Wo = np.asarray(inputs["Wo"], dtype=np.float32)
    bo = np.asarray(inputs["bo"], dtype=np.float32)

    bf = ml_dtypes.bfloat16
    # rope tables (from the position_ids input)
    p = pos[0].astype(np.float32)
    inv = 1.0 / (10000.0 ** (np.arange(0, HD, 2, dtype=np.float32) / HD))
    fr = p[:, None] * inv[None, :]                       # (S, 32)
    emb = np.concatenate([fr, fr], axis=1)               # (S, 64)
    cosT = np.cos(emb).T.astype(np.float32)              # (64, S)
    sinT = np.sin(emb).T.astype(np.float32)
    # pre-shifted signed sin: multiplied at src rows, then shifted to dst
    ss_pre = np.concatenate([sinT[32:64], -sinT[0:32]], axis=0)  # (64, S)
    kcos = np.tile(cosT, (2, 1)).astype(bf)
    ksin = np.tile(ss_pre, (2, 1)).astype(bf)

    # all causal diagonal blocks are identical; ship one
    maskd = np.ascontiguousarray(mask[0, 0, 0:128, 0:128].T)

    # fold V bias into the output bias: softmax rows sum to 1, so the v-bias
    # contributes exactly Wo @ repeat_kv(bv) to every position.
    bv_full = np.concatenate(
        [bv[64 * (i // 4):64 * (i // 4) + 64] for i in range(NH)])
    bo_fold = (bo.astype(np.float64)
               + Wo.astype(np.float64) @ bv_full.astype(np.float64)
               ).astype(np.float32)

    def packw(wslice_t):
        # [H, cols] -> [128, HT*cols]: row-tiles side by side
        return np.ascontiguousarray(np.concatenate(
            [wslice_t[128 * t:128 * (t + 1)] for t in range(HT)], axis=1))

    in_maps = []
    for c in range(N_CORES):
        b, hg = c // 4, c % 4
        in_maps.append({
            "xt": np.ascontiguousarray(x[b].T).astype(bf),
            "wqt": packw(Wq[512 * hg:512 * (hg + 1), :].T * SCALE).astype(bf),
            "wkt": packw(Wk[128 * hg:128 * (hg + 1), :].T).astype(bf),
            "wvt": packw(Wv[128 * hg:128 * (hg + 1), :].T).astype(bf),
            "wot": packw(Wo[512 * hg:512 * (hg + 1), :].T).astype(bf),
            "bq": np.ascontiguousarray(bq[512 * hg:512 * (hg + 1)] * SCALE)[:, None],
            "bk": np.ascontiguousarray(bk[128 * hg:128 * (hg + 1)])[:, None],
            "bo": np.ascontiguousarray(bo_fold[512 * hg:512 * (hg + 1)])[:, None],
            "kcos": kcos, "ksin": ksin,
            "maskd": maskd,
        })

    if _COMPILED is None:
        _install_profile_shim()
        _COMPILED = _build()

    res = bass_utils.run_bass_kernel_spmd(
        _COMPILED, in_maps, core_ids=list(range(N_CORES)), trace=TRACE)
    LAST_EXEC_NS = res.exec_time_ns

    outb = []
    for b in range(B):
        yt = np.concatenate([res.results[4 * b + hg]["out"]
                             for hg in range(4)], axis=0)   # [2048 oc, 2048 q]
        outb.append(yt.T)
    return np.stack(outb).astype(np.float32)


# revision 20
# speedup vs baseline: 1.0195x; 1.0195x over previous
"""Distributed GQA attention (B=2, S=2048, H=2048, 32 heads / 8 KV heads,
RoPE, causal) on 8 TRN2 NeuronCores.

Sharding: core c -> (batch b = c//4, head-group hg = c%4).
Each core computes q-heads [8hg, 8hg+8) and kv-heads [2hg, 2hg+2) of its
batch, runs attention locally (GQA groups stay on-core), then the four
cores of a batch AllGather their attention outputs (bf16) and each
computes a disjoint 512-column slice of the output projection, so no
all-reduce is needed.  Host reassembles the 8 disjoint slices.

Device layouts are transposed ([channel, row]) so RoPE / QK / O-proj
contract along partitions; softmax runs without max-subtraction
(scores are bounded) and denominators come from a ones-column appended
to V.  AV is computed with exp-scores as the stationary operand so the
output lands [q-partitions, head-dim] and the softmax denominator is a
per-partition column (cheap reciprocal + per-partition scale), then
PE-transposed back to [channel, q].  The V bias is folded into the
output bias on host (softmax rows sum to 1).  Attention is split into
7 column phases; each phase's AllGather overlaps later attention and
its out-proj slice is deferred three phases (absorbing cross-core
launch skew at the first collective) and interleaved into later
attention as PE filler, as are the V-projection tiles.
"""
import os
import sys

sys.path.insert(0, "/opt/trn_rl_repo")

import numpy as np
import ml_dtypes

import concourse.bass as bass
import concourse.mybir as mybir
import concourse.tile as tile
from concourse import bacc
from concourse import bass_utils
from concourse.masks import make_identity

BF16 = mybir.dt.bfloat16
F32 = mybir.dt.float32
ADD = mybir.AluOpType.add
MULT = mybir.AluOpType.mult

B, S, H = 2, 2048, 2048
NH, NKV, HD = 32, 8, 64
SCALE = HD ** -0.5
RG = [[0, 1, 2, 3], [4, 5, 6, 7]]
N_CORES = 8
NT = S // 128          # 16 seq tiles
HT = H // 128          # 16 hidden tiles

# attention phases: qt ranges (balanced by causal work)
PH = [(0, 4), (4, 8), (8, 11), (11, 13), (13, 14), (14, 15), (15, 16)]
PC0 = [128 * lo for lo, _ in PH]           # column offset per phase
PW = [128 * (hi - lo) for lo, hi in PH]    # column width per phase

TRACE = os.environ.get("BASS_KERNEL_TRACE", "0") == "1"
LAST_EXEC_NS = None
_COMPILED = None


def _install_profile_shim():
    import types
    try:
        from trn_agent_boot.trn_boot import _ntff_profile_via_ctypes
    except ImportError:
        return
    hook = _ntff_profile_via_ctypes("/opt/axon/libaxon_pjrt.so")
    mod = types.ModuleType("antenv.axon_hooks")
    mod.get_axon_ntff_profile_hook = lambda: hook
    mod.set_axon_ntff_profile_hook = lambda h: None
    sys.modules["antenv.axon_hooks"] = mod
    bass_utils.upload_artifacts = lambda tmpdir: tmpdir


def _build():
    nc = bacc.Bacc("TRN2", target_bir_lowering=False, debug=False,
                   num_devices=N_CORES)

    xt = nc.dram_tensor("xt", [H, S], BF16, kind="ExternalInput")
    wqt = nc.dram_tensor("wqt", [128, HT * 512], BF16, kind="ExternalInput")
    wkt = nc.dram_tensor("wkt", [128, HT * 128], BF16, kind="ExternalInput")
    wvt = nc.dram_tensor("wvt", [128, HT * 128], BF16, kind="ExternalInput")
    wot = nc.dram_tensor("wot", [128, HT * 512], BF16, kind="ExternalInput")
    bq = nc.dram_tensor("bq", [512, 1], F32, kind="ExternalInput")
    bk = nc.dram_tensor("bk", [128, 1], F32, kind="ExternalInput")
    bo = nc.dram_tensor("bo", [512, 1], F32, kind="ExternalInput")
    kcos = nc.dram_tensor("kcos", [128, S], BF16, kind="ExternalInput")
    ksin = nc.dram_tensor("ksin", [128, S], BF16, kind="ExternalInput")
    maskd = nc.dram_tensor("maskd", [128, 128], F32, kind="ExternalInput")
    out = nc.dram_tensor("out", [512, S], F32, kind="ExternalOutput")

    Exp = mybir.ActivationFunctionType.Exp

    from contextlib import ExitStack
    with tile.TileContext(nc) as tc:
        with ExitStack() as stk:
            ep = stk.enter_context
            big = ep(tc.tile_pool(name="big", bufs=16))     # xt / gathered o
            wpool = ep(tc.tile_pool(name="w", bufs=2))      # wqt / wot
            wkpool = ep(tc.tile_pool(name="wk", bufs=1))
            wvpool = ep(tc.tile_pool(name="wv", bufs=1))
            qpool = ep(tc.tile_pool(name="qt", bufs=4))
            kpool = ep(tc.tile_pool(name="kt", bufs=2))
            vpool = ep(tc.tile_pool(name="vv", bufs=16))
            opool = ep(tc.tile_pool(name="ot", bufs=4))
            tabpool = ep(tc.tile_pool(name="tab", bufs=4))
            mkpool = ep(tc.tile_pool(name="mk", bufs=1))
            ropepool = ep(tc.tile_pool(name="rope", bufs=6))
            expool = ep(tc.tile_pool(name="exp", bufs=6))
            nrmpool = ep(tc.tile_pool(name="nrm", bufs=4))
            ypool = ep(tc.tile_pool(name="yy", bufs=2))
            bpool = ep(tc.tile_pool(name="bias", bufs=12))
            idpool = ep(tc.tile_pool(name="id", bufs=1))
            pp = ep(tc.tile_pool(name="pp", bufs=1, space="PSUM"))
            scp = ep(tc.tile_pool(name="sc", bufs=2, space="PSUM"))
            avp = ep(tc.tile_pool(name="av", bufs=2, space="PSUM"))
            tpp = ep(tc.tile_pool(name="tp", bufs=1, space="PSUM"))
            dram = ep(tc.tile_pool(name="dram", bufs=1, space="DRAM"))

            # ---------- input loads: small weights first, then xt ----------
            wk_sb = wkpool.tile([128, HT * 128], BF16, name="wk", tag="wk")
            nc.sync.dma_start(out=wk_sb[:, 0:1024], in_=wkt[:, 0:1024])
            nc.sync.dma_start(out=wk_sb[:, 1024:2048], in_=wkt[:, 1024:2048])
            wv_sb = wvpool.tile([128, HT * 128], BF16, name="wv", tag="wv")
            nc.sync.dma_start(out=wv_sb[:, 0:1024], in_=wvt[:, 0:1024])
            nc.sync.dma_start(out=wv_sb[:, 1024:2048], in_=wvt[:, 1024:2048])
            bq_sb, bo_sb = [], []
            for o in range(4):
                b_t = bpool.tile([128, 1], F32, name=f"bq{o}", tag="bias")
                nc.sync.dma_start(out=b_t[:, :], in_=bq[128 * o:128 * (o + 1), :])
                bq_sb.append(b_t)
            bk_sb = bpool.tile([128, 1], F32, name="bk", tag="bias")
            nc.sync.dma_start(out=bk_sb[:, :], in_=bk[:, :])
            for o in range(4):
                b_t = bpool.tile([128, 1], F32, name=f"bo{o}", tag="bias")
                nc.sync.dma_start(out=b_t[:, :], in_=bo[128 * o:128 * (o + 1), :])
                bo_sb.append(b_t)
            ident = idpool.tile([128, 128], BF16, name="ident", tag="id")
            make_identity(nc, ident[:, :])
            xt_sb = []
            for t in range(HT):
                x_t = big.tile([128, S], BF16, name=f"xt{t}", tag="big")
                nc.sync.dma_start(out=x_t[:, 0:1024],
                                  in_=xt[128 * t:128 * (t + 1), 0:1024])
                nc.sync.dma_start(out=x_t[:, 1024:2048],
                                  in_=xt[128 * t:128 * (t + 1), 1024:2048])
                xt_sb.append(x_t)
            wq_sb = wpool.tile([128, HT * 512], BF16, name="wq", tag="w")
            nc.sync.dma_start(out=wq_sb[:, :], in_=wqt[:, :])
            kcos_sb = tabpool.tile([128, S], BF16, name="kcos", tag="tab")
            nc.sync.dma_start(out=kcos_sb[:, :], in_=kcos[:, :])
            ksin_sb = tabpool.tile([128, S], BF16, name="ksin", tag="tab")
            nc.sync.dma_start(out=ksin_sb[:, :], in_=ksin[:, :])
            maskd_sb = mkpool.tile([128, 128], F32, name="maskd", tag="mk")
            nc.sync.dma_start(out=maskd_sb[:, :], in_=maskd[:, :])

            def rope(psum, bias_ap, cos_sb, sin_sb, c, out_ap):
                """out = (psum+bias)*cos + shift32((psum+bias)*sin_pre).

                The psum+bias runs on the scalar engine so the projection
                PSUM slot is released after one short op."""
                cs = slice(512 * c, 512 * (c + 1))
                tb = ropepool.tile([128, 512], F32, name="tb", tag="rope")
                nc.scalar.activation(tb[:, :], psum[:, :],
                                     mybir.ActivationFunctionType.Identity,
                                     bias=bias_ap)
                tcos = ropepool.tile([128, 512], F32, name="tcos", tag="rope")
                nc.vector.tensor_tensor(tcos[:, :], tb[:, :], cos_sb[:, cs],
                                        MULT)
                tsin = ropepool.tile([128, 512], F32, name="tsin", tag="rope")
                nc.vector.tensor_tensor(tsin[:, :], tb[:, :], sin_sb[:, cs],
                                        MULT)
                tsh = ropepool.tile([128, 512], F32, name="tsh", tag="rope")
                for d, s in ((0, 32), (32, 0), (64, 96), (96, 64)):
                    nc.sync.dma_start(out=tsh[d:d + 32, :], in_=tsin[s:s + 32, :])
                nc.vector.tensor_tensor(out_ap, tcos[:, :], tsh[:, :], ADD)

            # ---------- K projection + rope ----------
            kT_sb = kpool.tile([128, S], BF16, name="kT", tag="kt")
            kT_sw = kpool.tile([128, S], BF16, name="kTswap", tag="kt")
            for c in range(4):
                ps = pp.tile([128, 512], F32, name="psk", tag="pp")
                for t in range(HT):
                    nc.tensor.matmul(ps[:, :],
                                     wk_sb[:, 128 * t:128 * (t + 1)],
                                     xt_sb[t][:, 512 * c:512 * (c + 1)],
                                     start=(t == 0), stop=(t == HT - 1))
                rope(ps, bk_sb[:, :], kcos_sb, ksin_sb, c,
                     kT_sb[:, 512 * c:512 * (c + 1)])
            # kT_sw: swapped kv halves (kv1 on partitions 0:64, kv0 on 64:128)
            nc.sync.dma_start(out=kT_sw[0:64, :], in_=kT_sb[64:128, :])
            nc.sync.dma_start(out=kT_sw[64:128, :], in_=kT_sb[0:64, :])

            # ---------- V projection (layout [rows, oc], 65-strided + ones) ----------
            # emitted lazily: early tiles up front, the rest as filler
            # between attention phases (v[rt] is first read at qt == rt)
            v_sb = [vpool.tile([128, 130], BF16, name=f"v{rt}", tag="v")
                    for rt in range(NT)]

            def emit_vproj(rt):
                v_t = v_sb[rt]
                nc.gpsimd.memset(
                    v_t[:, :].rearrange("p (m c) -> p m c", c=65)[:, :, 64:65], 1.0)
                ps = pp.tile([128, 128], F32, name="psv", tag="pp")
                for t in range(HT):
                    nc.tensor.matmul(ps[:, :],
                                     xt_sb[t][:, 128 * rt:128 * (rt + 1)],
                                     wv_sb[:, 128 * t:128 * (t + 1)],
                                     start=(t == 0), stop=(t == HT - 1))
                nc.scalar.activation(
                    v_t[:, :].rearrange("p (m c) -> p m c", c=65)[:, :, 0:64],
                    ps[:, :].rearrange("p (m c) -> p m c", c=64),
                    mybir.ActivationFunctionType.Copy)

            for rt in range(4):
                emit_vproj(rt)

            # ---------- Q projection + rope (scale folded into weights) ----------
            # chunk c produces q columns [512c, 512c+512) = qt tiles 4c..4c+3,
            # so only c=0 must precede attention; later chunks are PE filler.
            qT_sb = [None] * 4

            def emit_qproj_chunk(o, c):
                if qT_sb[o] is None:
                    qT_sb[o] = qpool.tile([128, S], BF16, name=f"qT{o}",
                                          tag="qt")
                q_t = qT_sb[o]
                ps = pp.tile([128, 512], F32, name="psq", tag="pp")
                for t in range(HT):
                    nc.tensor.matmul(ps[:, :],
                                     wq_sb[:, 512 * t + 128 * o:
                                           512 * t + 128 * (o + 1)],
                                     xt_sb[t][:, 512 * c:512 * (c + 1)],
                                     start=(t == 0), stop=(t == HT - 1))
                rope(ps, bq_sb[o][:, :], kcos_sb, ksin_sb, c,
                     q_t[:, 512 * c:512 * (c + 1)])

            # ---------- attention ----------
            oT_sb = []
            for j in range(4):
                o_t = opool.tile([128, S], BF16, name=f"oT{j}", tag="ot")
                oT_sb.append(o_t)

            NPH = len(PH)
            bounce = [dram.tile([512, PW[p]], BF16, name=f"bounce{p}",
                                tag=f"bounce{p}") for p in range(NPH)]
            gath = [dram.tile([4, 512, PW[p]], BF16, name=f"gath{p}",
                              tag=f"gath{p}") for p in range(NPH)]
            gsb_all = [None] * NPH
            wo_sb = [None]

            def emit_attn_chain(j, qt):
                """One (head-pair, q-tile) attention chain; chains from
                different j are independent and pipeline through PE/scalar."""
                kvl = j // 2
                ke = kT_sb if kvl == 0 else kT_sw
                ko = kT_sw if kvl == 0 else kT_sb
                qs = slice(128 * qt, 128 * (qt + 1))
                av_e = avp.tile([128, 65], F32, name="ave", tag="av")
                av_o = avp.tile([128, 65], F32, name="avo", tag="av")

                def emit_scores(k0, width):
                    sc2 = scp.tile([128, 1024], F32, name="sc2", tag="sc")
                    for cc in range(width):
                        kt = k0 + cc
                        cs = slice(128 * cc, 128 * (cc + 1))
                        co = slice(512 + 128 * cc, 512 + 128 * (cc + 1))
                        ks = slice(128 * kt, 128 * (kt + 1))
                        nc.tensor.matmul(sc2[:, cs], ke[0:64, ks],
                                         qT_sb[j][0:64, qs],
                                         start=True, stop=True,
                                         tile_position=(0, 0))
                        nc.tensor.matmul(sc2[:, co], ko[64:128, ks],
                                         qT_sb[j][64:128, qs],
                                         start=True, stop=True,
                                         tile_position=(64, 0))
                        if kt == qt:
                            nc.vector.tensor_tensor(sc2[:, cs], sc2[:, cs],
                                                    maskd_sb[:, :], ADD)
                            nc.vector.tensor_tensor(sc2[:, co], sc2[:, co],
                                                    maskd_sb[:, :], ADD)
                    return sc2

                def emit_expav(sc2, k0, width):
                    w = 128 * width
                    eb2 = expool.tile([128, 1024], BF16, name="eb2", tag="exp")
                    nc.scalar.activation(eb2[:, 0:w], sc2[:, 0:w], Exp)
                    nc.scalar.activation(eb2[:, 512:512 + w],
                                         sc2[:, 512:512 + w], Exp)
                    for par, base in ((0, 0), (1, 512)):
                        avt = (av_e if par == 0 else av_o)[:, :]
                        for cc in range(width):
                            ktt = k0 + cc
                            vs = v_sb[ktt][:, 65 * kvl:65 * kvl + 65]
                            ccs = slice(base + 128 * cc, base + 128 * (cc + 1))
                            nc.tensor.matmul(avt, eb2[:, ccs], vs,
                                             start=(ktt == 0),
                                             stop=(ktt == qt))

                chunks = [(k0, min(4, qt + 1 - k0))
                          for k0 in range(0, qt + 1, 4)]
                prev = None
                for ch in chunks:
                    sc2 = emit_scores(*ch)
                    if prev is not None:
                        emit_expav(*prev)
                    prev = (sc2, *ch)
                emit_expav(*prev)
                # normalize ([q, d] layout: denom is a per-partition col),
                # then PE-transpose back to [d, q]
                ps_t = tpp.tile([128, 128], BF16, name="pst", tag="tp")
                for par, av in ((0, av_e), (1, av_o)):
                    rc = nrmpool.tile([128, 1], F32, name="rc", tag="rc")
                    nc.vector.reciprocal(rc[:, :], av[:, 64:65])
                    nm = nrmpool.tile([128, 64], BF16, name="nm", tag="nm")
                    nc.vector.tensor_scalar_mul(nm[:, :], av[:, 0:64],
                                                rc[:, :])
                    nc.tensor.transpose(ps_t[64 * par:64 * (par + 1), :],
                                        nm[:, :], ident[:, :])
                nc.vector.tensor_copy(oT_sb[j][:, qs], ps_t[:, :])

            def emit_gather(p):
                for jj in range(4):
                    nc.sync.dma_start(
                        out=bounce[p][128 * jj:128 * (jj + 1), :],
                        in_=oT_sb[jj][:, PC0[p]:PC0[p] + PW[p]])
                nc.gpsimd.collective_compute(
                    "AllGather", mybir.AluOpType.bypass, replica_groups=RG,
                    ins=[bounce[p][:, :].opt()],
                    outs=[gath[p][:, :, :].opt()])
                # prefetch the gathered tiles while later attention runs
                W = PW[p]
                gview = gath[p][:, :, :].rearrange("g i q -> (g i) q")
                gsb = []
                for t in range(HT):
                    g_t = big.tile([128, W], BF16, name=f"g{p}_{t}", tag="big")
                    nc.sync.dma_start(out=g_t[:, :],
                                      in_=gview[128 * t:128 * (t + 1), :])
                    gsb.append(g_t)
                gsb_all[p] = gsb

            def emit_wo_loads():
                w_t = wpool.tile([128, HT * 512], BF16, name="wo", tag="w")
                nc.sync.dma_start(out=w_t[:, :], in_=wot[:, :])
                wo_sb[0] = w_t

            def oproj_chunks(p):
                W = PW[p]
                gsb = gsb_all[p]
                for o in range(4):
                    for q0 in range(0, W, 512):
                        yield (p, o, q0, min(512, W - q0), gsb)

            def emit_oproj_chunk(chunk):
                p, o, q0, cw, gsb = chunk
                ps = pp.tile([128, cw], F32, name="psy", tag="pp")
                for t in range(HT):
                    nc.tensor.matmul(
                        ps[:, :],
                        wo_sb[0][:, 512 * t + 128 * o:
                                 512 * t + 128 * (o + 1)],
                        gsb[t][:, q0:q0 + cw],
                        start=(t == 0), stop=(t == HT - 1))
                y_t = ypool.tile([128, cw], F32, name="y", tag="y")
                nc.vector.tensor_scalar_add(y_t[:, :], ps[:, :],
                                            bo_sb[o][:, :])
                nc.sync.dma_start(
                    out=out[128 * o:128 * (o + 1),
                            PC0[p] + q0:PC0[p] + q0 + cw],
                    in_=y_t[:, :])

            def emit_oproj(p):
                for ch in oproj_chunks(p):
                    emit_oproj_chunk(ch)

            # phase 0 interleaved with Q projection; later Q chunks and
            # late V tiles interleave into phases 0-2 as PE filler
            for j in range(4):
                emit_qproj_chunk(j, 0)
                for qt in range(*PH[0]):
                    emit_attn_chain(j, qt)
                emit_qproj_chunk(j, 1)
                emit_vproj(4 + j)
            emit_wo_loads()
            emit_gather(0)
            for j in range(4):
                for qt in range(*PH[1]):
                    emit_attn_chain(j, qt)
                emit_qproj_chunk(j, 2)
                if 8 + j < NT:
                    emit_vproj(8 + j)
            emit_gather(1)
            for j in range(4):
                for qt in range(*PH[2]):
                    emit_attn_chain(j, qt)
                emit_qproj_chunk(j, 3)
                if 12 + j < NT:
                    emit_vproj(12 + j)
            emit_gather(2)
            for p in range(3, NPH):
                pending = list(oproj_chunks(p - 3))
                nchunk = len(pending)
                for j in range(4):
                    for qt in range(*PH[p]):
                        emit_attn_chain(j, qt)
                    for ch in pending[(nchunk * j + 3) // 4:
                                      (nchunk * (j + 1) + 3) // 4]:
                        emit_oproj_chunk(ch)
                emit_gather(p)
            emit_oproj(NPH - 3)
            emit_oproj(NPH - 2)
            emit_oproj(NPH - 1)

    nc.compile()
    return nc


def kernel(**inputs):
    global _COMPILED, LAST_EXEC_NS
    x = np.asarray(inputs["hidden_states"], dtype=np.float32)
    mask = np.asarray(inputs["attention_mask"], dtype=np.float32)
    pos = np.asarray(inputs["position_ids"])
    Wq = np.asarray(inputs["Wq"], dtype=np.float32)
    bq = np.asarray(inputs["bq"], dtype=np.float32)
    Wk = np.asarray(inputs["Wk"], dtype=np.float32)
    bk = np.asarray(inputs["bk"], dtype=np.float32)
    Wv = np.asarray(inputs["Wv"], dtype=np.float32)
    bv = np.asarray(inputs["bv"], dtype=np.float32)
    Wo = np.asarray(inputs["Wo"], dtype=np.float32)
    bo = np.asarray(inputs["bo"], dtype=np.float32)

    bf = ml_dtypes.bfloat16
    # rope tables (from the position_ids input)
    p = pos[0].astype(np.float32)
    inv = 1.0 / (10000.0 ** (np.arange(0, HD, 2, dtype=np.float32) / HD))
    fr = p[:, None] * inv[None, :]                       # (S, 32)
    emb = np.concatenate([fr, fr], axis=1)               # (S, 64)
    cosT = np.cos(emb).T.astype(np.float32)              # (64, S)
    sinT = np.sin(emb).T.astype(np.float32)
    # pre-shifted signed sin: multiplied at src rows, then shifted to dst
    ss_pre = np.concatenate([sinT[32:64], -sinT[0:32]], axis=0)  # (64, S)
    kcos = np.tile(cosT, (2, 1)).astype(bf)
    ksin = np.tile(ss_pre, (2, 1)).astype(bf)

    # all causal diagonal blocks are identical; ship one
    maskd = np.ascontiguousarray(mask[0, 0, 0:128, 0:128].T)

    # fold V bias into the output bias: softmax rows sum to 1, so the v-bias
    # contributes exactly Wo @ repeat_kv(bv) to every position.
    bv_full = np.concatenate(
        [bv[64 * (i // 4):64 * (i // 4) + 64] for i in range(NH)])
    bo_fold = (bo.astype(np.float64)
               + Wo.astype(np.float64) @ bv_full.astype(np.float64)
               ).astype(np.float32)

    def packw(wslice_t):
        # [H, cols] -> [128, HT*cols]: row-tiles side by side
        return np.ascontiguousarray(np.concatenate(
            [wslice_t[128 * t:128 * (t + 1)] for t in range(HT)], axis=1))

    in_maps = []
    for c in range(N_CORES):
        b, hg = c // 4, c % 4
        in_maps.append({
            "xt": np.ascontiguousarray(x[b].T).astype(bf),
            "wqt": packw(Wq[512 * hg:512 * (hg + 1), :].T * SCALE).astype(bf),
            "wkt": packw(Wk[128 * hg:128 * (hg + 1), :].T).astype(bf),
            "wvt": packw(Wv[128 * hg:128 * (hg + 1), :].T).astype(bf),
            "wot": packw(Wo[512 * hg:512 * (hg + 1), :].T).astype(bf),
            "bq": np.ascontiguousarray(bq[512 * hg:512 * (hg + 1)] * SCALE)[:, None],
            "bk": np.ascontiguousarray(bk[128 * hg:128 * (hg + 1)])[:, None],
            "bo": np.ascontiguousarray(bo_fold[512 * hg:512 * (hg + 1)])[:, None],
            "kcos": kcos, "ksin": ksin,
            "maskd": maskd,
        })

    if _COMPILED is None:
        _install_profile_shim()
        _COMPILED = _build()

    res = bass_utils.run_bass_kernel_spmd(
        _COMPILED, in_maps, core_ids=list(range(N_CORES)), trace=TRACE)
    LAST_EXEC_NS = res.exec_time_ns

    outb = []
    for b in range(B):
        yt = np.concatenate([res.results[4 * b + hg]["out"]
                             for hg in range(4)], axis=0)   # [2048 oc, 2048 q]
        outb.append(yt.T)
    return np.stack(outb).astype(np.float32)


# revision 21
# speedup vs baseline: 1.1281x; 1.1065x over previous
"""Distributed GQA attention (B=2, S=2048, H=2048, 32 heads / 8 KV heads,
RoPE, causal) on 8 TRN2 NeuronCores.

Sharding: core c -> (batch b = c//4, head-group hg = c%4).
Each core computes q-heads [8hg, 8hg+8) and kv-heads [2hg, 2hg+2) of its
batch, runs attention locally (GQA groups stay on-core), then the four
cores of a batch AllGather their attention outputs (bf16) and each
computes a disjoint 512-column slice of the output projection, so no
all-reduce is needed.  Host reassembles the 8 disjoint slices.

Device layouts are transposed ([channel, row]) so RoPE / QK / O-proj
contract along partitions; softmax runs without max-subtraction
(scores are bounded) and denominators come from a ones-column appended
to V.  AV is computed with exp-scores as the stationary operand so the
output lands [q-partitions, head-dim] and the softmax denominator is a
per-partition column (cheap reciprocal + per-partition scale), then
PE-transposed back to [channel, q].  The V bias is folded into the
output bias on host (softmax rows sum to 1).  Attention is split into
7 column phases; each phase's AllGather overlaps later attention and
its out-proj slice is deferred three phases (absorbing cross-core
launch skew at the first collective) and interleaved into later
attention as PE filler, as are the V-projection tiles.
"""
import os
import sys

sys.path.insert(0, "/opt/trn_rl_repo")

import numpy as np
import ml_dtypes

import concourse.bass as bass
import concourse.mybir as mybir
import concourse.tile as tile
from concourse import bacc
from concourse import bass_utils
from concourse.masks import make_identity

BF16 = mybir.dt.bfloat16
F32 = mybir.dt.float32
ADD = mybir.AluOpType.add
MULT = mybir.AluOpType.mult

B, S, H = 2, 2048, 2048
NH, NKV, HD = 32, 8, 64
SCALE = HD ** -0.5
RG = [[0, 1, 2, 3], [4, 5, 6, 7]]
N_CORES = 8
NT = S // 128          # 16 seq tiles
HT = H // 128          # 16 hidden tiles

# attention phases: qt ranges (balanced by causal work)
PH = [(0, 4), (4, 8), (8, 11), (11, 13), (13, 14), (14, 15), (15, 16)]
PC0 = [128 * lo for lo, _ in PH]           # column offset per phase
PW = [128 * (hi - lo) for lo, hi in PH]    # column width per phase

TRACE = os.environ.get("BASS_KERNEL_TRACE", "0") == "1"
LAST_EXEC_NS = None
_COMPILED = None


def _install_profile_shim():
    import types
    try:
        from trn_agent_boot.trn_boot import _ntff_profile_via_ctypes
    except ImportError:
        return
    hook = _ntff_profile_via_ctypes("/opt/axon/libaxon_pjrt.so")
    mod = types.ModuleType("antenv.axon_hooks")
    mod.get_axon_ntff_profile_hook = lambda: hook
    mod.set_axon_ntff_profile_hook = lambda h: None
    sys.modules["antenv.axon_hooks"] = mod
    bass_utils.upload_artifacts = lambda tmpdir: tmpdir


def _build():
    nc = bacc.Bacc("TRN2", target_bir_lowering=False, debug=False,
                   num_devices=N_CORES)

    xt = nc.dram_tensor("xt", [H, S], BF16, kind="ExternalInput")
    wqt = nc.dram_tensor("wqt", [128, HT * 512], BF16, kind="ExternalInput")
    wkt = nc.dram_tensor("wkt", [128, HT * 128], BF16, kind="ExternalInput")
    wvt = nc.dram_tensor("wvt", [128, HT * 128], BF16, kind="ExternalInput")
    wot = nc.dram_tensor("wot", [128, HT * 512], BF16, kind="ExternalInput")
    bq = nc.dram_tensor("bq", [512, 1], F32, kind="ExternalInput")
    bk = nc.dram_tensor("bk", [128, 1], F32, kind="ExternalInput")
    bo = nc.dram_tensor("bo", [512, 1], F32, kind="ExternalInput")
    kcos = nc.dram_tensor("kcos", [128, S], BF16, kind="ExternalInput")
    ksin = nc.dram_tensor("ksin", [128, S], BF16, kind="ExternalInput")
    maskd = nc.dram_tensor("maskd", [128, 128], F32, kind="ExternalInput")
    out = nc.dram_tensor("out", [512, S], F32, kind="ExternalOutput")

    Exp = mybir.ActivationFunctionType.Exp

    from contextlib import ExitStack
    with tile.TileContext(nc) as tc:
        with ExitStack() as stk:
            ep = stk.enter_context
            big = ep(tc.tile_pool(name="big", bufs=16))     # xt / gathered o
            wpool = ep(tc.tile_pool(name="w", bufs=2))      # wqt / wot
            wkpool = ep(tc.tile_pool(name="wk", bufs=1))
            wvpool = ep(tc.tile_pool(name="wv", bufs=1))
            qpool = ep(tc.tile_pool(name="qt", bufs=4))
            kpool = ep(tc.tile_pool(name="kt", bufs=2))
            vpool = ep(tc.tile_pool(name="vv", bufs=16))
            opool = ep(tc.tile_pool(name="ot", bufs=4))
            tabpool = ep(tc.tile_pool(name="tab", bufs=4))
            mkpool = ep(tc.tile_pool(name="mk", bufs=1))
            ropepool = ep(tc.tile_pool(name="rope", bufs=6))
            expool = ep(tc.tile_pool(name="exp", bufs=6))
            nrmpool = ep(tc.tile_pool(name="nrm", bufs=4))
            ypool = ep(tc.tile_pool(name="yy", bufs=2))
            bpool = ep(tc.tile_pool(name="bias", bufs=12))
            idpool = ep(tc.tile_pool(name="id", bufs=1))
            pp = ep(tc.tile_pool(name="pp", bufs=1, space="PSUM"))
            scp = ep(tc.tile_pool(name="sc", bufs=2, space="PSUM"))
            avp = ep(tc.tile_pool(name="av", bufs=2, space="PSUM"))
            tpp = ep(tc.tile_pool(name="tp", bufs=1, space="PSUM"))
            dram = ep(tc.tile_pool(name="dram", bufs=1, space="DRAM"))

            # ---------- input loads: small weights first, then xt ----------
            wk_sb = wkpool.tile([128, HT * 128], BF16, name="wk", tag="wk")
            nc.sync.dma_start(out=wk_sb[:, 0:1024], in_=wkt[:, 0:1024])
            nc.sync.dma_start(out=wk_sb[:, 1024:2048], in_=wkt[:, 1024:2048])
            wv_sb = wvpool.tile([128, HT * 128], BF16, name="wv", tag="wv")
            nc.sync.dma_start(out=wv_sb[:, 0:1024], in_=wvt[:, 0:1024])
            nc.sync.dma_start(out=wv_sb[:, 1024:2048], in_=wvt[:, 1024:2048])
            bq_sb, bo_sb = [], []
            for o in range(4):
                b_t = bpool.tile([128, 1], F32, name=f"bq{o}", tag="bias")
                nc.sync.dma_start(out=b_t[:, :], in_=bq[128 * o:128 * (o + 1), :])
                bq_sb.append(b_t)
            bk_sb = bpool.tile([128, 1], F32, name="bk", tag="bias")
            nc.sync.dma_start(out=bk_sb[:, :], in_=bk[:, :])
            for o in range(4):
                b_t = bpool.tile([128, 1], F32, name=f"bo{o}", tag="bias")
                nc.sync.dma_start(out=b_t[:, :], in_=bo[128 * o:128 * (o + 1), :])
                bo_sb.append(b_t)
            ident = idpool.tile([128, 128], BF16, name="ident", tag="id")
            make_identity(nc, ident[:, :])
            xt_sb = []
            for t in range(HT):
                x_t = big.tile([128, S], BF16, name=f"xt{t}", tag="big")
                nc.sync.dma_start(out=x_t[:, 0:1024],
                                  in_=xt[128 * t:128 * (t + 1), 0:1024])
                nc.sync.dma_start(out=x_t[:, 1024:2048],
                                  in_=xt[128 * t:128 * (t + 1), 1024:2048])
                xt_sb.append(x_t)
            wq_sb = wpool.tile([128, HT * 512], BF16, name="wq", tag="w")
            nc.sync.dma_start(out=wq_sb[:, :], in_=wqt[:, :])
            kcos_sb = tabpool.tile([128, S], BF16, name="kcos", tag="tab")
            nc.sync.dma_start(out=kcos_sb[:, :], in_=kcos[:, :])
            ksin_sb = tabpool.tile([128, S], BF16, name="ksin", tag="tab")
            nc.sync.dma_start(out=ksin_sb[:, :], in_=ksin[:, :])
            maskd_sb = mkpool.tile([128, 128], F32, name="maskd", tag="mk")
            nc.sync.dma_start(out=maskd_sb[:, :], in_=maskd[:, :])

            def rope(psum, bias_ap, cos_sb, sin_sb, c, out_ap):
                """out = (psum+bias)*cos + shift32((psum+bias)*sin_pre).

                The psum+bias runs on the scalar engine so the projection
                PSUM slot is released after one short op."""
                cs = slice(512 * c, 512 * (c + 1))
                tb = ropepool.tile([128, 512], F32, name="tb", tag="rope")
                nc.scalar.activation(tb[:, :], psum[:, :],
                                     mybir.ActivationFunctionType.Identity,
                                     bias=bias_ap)
                tcos = ropepool.tile([128, 512], F32, name="tcos", tag="rope")
                nc.vector.tensor_tensor(tcos[:, :], tb[:, :], cos_sb[:, cs],
                                        MULT)
                tsin = ropepool.tile([128, 512], F32, name="tsin", tag="rope")
                nc.vector.tensor_tensor(tsin[:, :], tb[:, :], sin_sb[:, cs],
                                        MULT)
                tsh = ropepool.tile([128, 512], F32, name="tsh", tag="rope")
                for d, s in ((0, 32), (32, 0), (64, 96), (96, 64)):
                    nc.sync.dma_start(out=tsh[d:d + 32, :], in_=tsin[s:s + 32, :])
                nc.vector.tensor_tensor(out_ap, tcos[:, :], tsh[:, :], ADD)

            # ---------- K projection + rope ----------
            kT_sb = kpool.tile([128, S], BF16, name="kT", tag="kt")
            kT_sw = kpool.tile([128, S], BF16, name="kTswap", tag="kt")
            for c in range(4):
                ps = pp.tile([128, 512], F32, name="psk", tag="pp")
                for t in range(HT):
                    nc.tensor.matmul(ps[:, :],
                                     wk_sb[:, 128 * t:128 * (t + 1)],
                                     xt_sb[t][:, 512 * c:512 * (c + 1)],
                                     start=(t == 0), stop=(t == HT - 1))
                rope(ps, bk_sb[:, :], kcos_sb, ksin_sb, c,
                     kT_sb[:, 512 * c:512 * (c + 1)])
            # kT_sw: swapped kv halves (kv1 on partitions 0:64, kv0 on 64:128)
            nc.sync.dma_start(out=kT_sw[0:64, :], in_=kT_sb[64:128, :])
            nc.sync.dma_start(out=kT_sw[64:128, :], in_=kT_sb[0:64, :])

            # ---------- V projection (layout [rows, oc], 65-strided + ones) ----------
            # emitted lazily: early tiles up front, the rest as filler
            # between attention phases (v[rt] is first read at qt == rt)
            v_sb = [vpool.tile([128, 130], BF16, name=f"v{rt}", tag="v")
                    for rt in range(NT)]

            def emit_vproj(rt):
                v_t = v_sb[rt]
                nc.gpsimd.memset(
                    v_t[:, :].rearrange("p (m c) -> p m c", c=65)[:, :, 64:65], 1.0)
                ps = pp.tile([128, 128], F32, name="psv", tag="pp")
                for t in range(HT):
                    nc.tensor.matmul(ps[:, :],
                                     xt_sb[t][:, 128 * rt:128 * (rt + 1)],
                                     wv_sb[:, 128 * t:128 * (t + 1)],
                                     start=(t == 0), stop=(t == HT - 1))
                nc.scalar.activation(
                    v_t[:, :].rearrange("p (m c) -> p m c", c=65)[:, :, 0:64],
                    ps[:, :].rearrange("p (m c) -> p m c", c=64),
                    mybir.ActivationFunctionType.Copy)

            for rt in range(4):
                emit_vproj(rt)

            # fine-grained PE filler: generators yield every few matmuls so
            # pieces can be interleaved INSIDE attention chains (PE queues
            # are in-order; only work emitted between a chain's chunks can
            # fill its exp-latency bubbles).  Only the head generator ever
            # advances, so single-slot pools never deadlock.
            fill_q = []

            def filler(n=1):
                for _ in range(n):
                    while fill_q:
                        try:
                            next(fill_q[0])
                            break
                        except StopIteration:
                            fill_q.pop(0)
                    if not fill_q:
                        return

            def drain_fill():
                while fill_q:
                    try:
                        next(fill_q[0])
                    except StopIteration:
                        fill_q.pop(0)

            # ---------- Q projection + rope (scale folded into weights) ----------
            # chunk c produces q columns [512c, 512c+512) = qt tiles 4c..4c+3,
            # so only c=0 must precede attention; later chunks are PE filler.
            qT_sb = [None] * 4

            def emit_qproj_chunk(o, c):
                if qT_sb[o] is None:
                    qT_sb[o] = qpool.tile([128, S], BF16, name=f"qT{o}",
                                          tag="qt")
                q_t = qT_sb[o]
                ps = pp.tile([128, 512], F32, name="psq", tag="pp")
                for t in range(HT):
                    nc.tensor.matmul(ps[:, :],
                                     wq_sb[:, 512 * t + 128 * o:
                                           512 * t + 128 * (o + 1)],
                                     xt_sb[t][:, 512 * c:512 * (c + 1)],
                                     start=(t == 0), stop=(t == HT - 1))
                rope(ps, bq_sb[o][:, :], kcos_sb, ksin_sb, c,
                     q_t[:, 512 * c:512 * (c + 1)])

            def qproj_gen(o, c):
                if qT_sb[o] is None:
                    qT_sb[o] = qpool.tile([128, S], BF16, name=f"qT{o}",
                                          tag="qt")
                q_t = qT_sb[o]
                ps = pp.tile([128, 512], F32, name="psq", tag="pp")
                for t0 in range(0, HT, 4):
                    for t in range(t0, t0 + 4):
                        nc.tensor.matmul(ps[:, :],
                                         wq_sb[:, 512 * t + 128 * o:
                                               512 * t + 128 * (o + 1)],
                                         xt_sb[t][:, 512 * c:512 * (c + 1)],
                                         start=(t == 0), stop=(t == HT - 1))
                    yield
                rope(ps, bq_sb[o][:, :], kcos_sb, ksin_sb, c,
                     q_t[:, 512 * c:512 * (c + 1)])

            def vproj_gen(rt):
                v_t = v_sb[rt]
                nc.gpsimd.memset(
                    v_t[:, :].rearrange("p (m c) -> p m c", c=65)[:, :, 64:65],
                    1.0)
                ps = pp.tile([128, 128], F32, name="psv", tag="pp")
                for t0 in range(0, HT, 4):
                    for t in range(t0, t0 + 4):
                        nc.tensor.matmul(ps[:, :],
                                         xt_sb[t][:, 128 * rt:128 * (rt + 1)],
                                         wv_sb[:, 128 * t:128 * (t + 1)],
                                         start=(t == 0), stop=(t == HT - 1))
                    yield
                nc.scalar.activation(
                    v_t[:, :].rearrange("p (m c) -> p m c", c=65)[:, :, 0:64],
                    ps[:, :].rearrange("p (m c) -> p m c", c=64),
                    mybir.ActivationFunctionType.Copy)

            # ---------- attention ----------
            oT_sb = []
            for j in range(4):
                o_t = opool.tile([128, S], BF16, name=f"oT{j}", tag="ot")
                oT_sb.append(o_t)

            NPH = len(PH)
            bounce = [dram.tile([512, PW[p]], BF16, name=f"bounce{p}",
                                tag=f"bounce{p}") for p in range(NPH)]
            gath = [dram.tile([4, 512, PW[p]], BF16, name=f"gath{p}",
                              tag=f"gath{p}") for p in range(NPH)]
            gsb_all = [None] * NPH
            wo_sb = [None]

            def emit_attn_chain(j, qt):
                """One (head-pair, q-tile) attention chain; chains from
                different j are independent and pipeline through PE/scalar."""
                kvl = j // 2
                ke = kT_sb if kvl == 0 else kT_sw
                ko = kT_sw if kvl == 0 else kT_sb
                qs = slice(128 * qt, 128 * (qt + 1))
                av_e = avp.tile([128, 65], F32, name="ave", tag="av")
                av_o = avp.tile([128, 65], F32, name="avo", tag="av")

                def emit_scores(k0, width):
                    sc2 = scp.tile([128, 1024], F32, name="sc2", tag="sc")
                    for cc in range(width):
                        kt = k0 + cc
                        cs = slice(128 * cc, 128 * (cc + 1))
                        co = slice(512 + 128 * cc, 512 + 128 * (cc + 1))
                        ks = slice(128 * kt, 128 * (kt + 1))
                        nc.tensor.matmul(sc2[:, cs], ke[0:64, ks],
                                         qT_sb[j][0:64, qs],
                                         start=True, stop=True,
                                         tile_position=(0, 0))
                        nc.tensor.matmul(sc2[:, co], ko[64:128, ks],
                                         qT_sb[j][64:128, qs],
                                         start=True, stop=True,
                                         tile_position=(64, 0))
                        if kt == qt:
                            nc.vector.tensor_tensor(sc2[:, cs], sc2[:, cs],
                                                    maskd_sb[:, :], ADD)
                            nc.vector.tensor_tensor(sc2[:, co], sc2[:, co],
                                                    maskd_sb[:, :], ADD)
                    return sc2

                def emit_expav(sc2, k0, width):
                    w = 128 * width
                    eb2 = expool.tile([128, 1024], BF16, name="eb2", tag="exp")
                    nc.scalar.activation(eb2[:, 0:w], sc2[:, 0:w], Exp)
                    nc.scalar.activation(eb2[:, 512:512 + w],
                                         sc2[:, 512:512 + w], Exp)
                    for par, base in ((0, 0), (1, 512)):
                        avt = (av_e if par == 0 else av_o)[:, :]
                        for cc in range(width):
                            ktt = k0 + cc
                            vs = v_sb[ktt][:, 65 * kvl:65 * kvl + 65]
                            ccs = slice(base + 128 * cc, base + 128 * (cc + 1))
                            nc.tensor.matmul(avt, eb2[:, ccs], vs,
                                             start=(ktt == 0),
                                             stop=(ktt == qt))

                chunks = [(k0, min(4, qt + 1 - k0))
                          for k0 in range(0, qt + 1, 4)]
                deep = len(chunks) > 2
                prev = None
                for ch in chunks:
                    sc2 = emit_scores(*ch)
                    filler(1 if deep else 2)
                    if prev is not None:
                        emit_expav(*prev)
                    prev = (sc2, *ch)
                filler(1)
                emit_expav(*prev)
                filler(1)
                # normalize ([q, d] layout: denom is a per-partition col),
                # then PE-transpose back to [d, q]
                ps_t = tpp.tile([128, 128], BF16, name="pst", tag="tp")
                for par, av in ((0, av_e), (1, av_o)):
                    rc = nrmpool.tile([128, 1], F32, name="rc", tag="rc")
                    nc.vector.reciprocal(rc[:, :], av[:, 64:65])
                    nm = nrmpool.tile([128, 64], BF16, name="nm", tag="nm")
                    nc.vector.tensor_scalar_mul(nm[:, :], av[:, 0:64],
                                                rc[:, :])
                    nc.tensor.transpose(ps_t[64 * par:64 * (par + 1), :],
                                        nm[:, :], ident[:, :])
                nc.vector.tensor_copy(oT_sb[j][:, qs], ps_t[:, :])

            def emit_gather(p):
                for jj in range(4):
                    nc.sync.dma_start(
                        out=bounce[p][128 * jj:128 * (jj + 1), :],
                        in_=oT_sb[jj][:, PC0[p]:PC0[p] + PW[p]])
                nc.gpsimd.collective_compute(
                    "AllGather", mybir.AluOpType.bypass, replica_groups=RG,
                    ins=[bounce[p][:, :].opt()],
                    outs=[gath[p][:, :, :].opt()])
                # prefetch the gathered tiles while later attention runs
                W = PW[p]
                gview = gath[p][:, :, :].rearrange("g i q -> (g i) q")
                gsb = []
                for t in range(HT):
                    g_t = big.tile([128, W], BF16, name=f"g{p}_{t}", tag="big")
                    nc.sync.dma_start(out=g_t[:, :],
                                      in_=gview[128 * t:128 * (t + 1), :])
                    gsb.append(g_t)
                gsb_all[p] = gsb

            def emit_wo_loads():
                w_t = wpool.tile([128, HT * 512], BF16, name="wo", tag="w")
                nc.sync.dma_start(out=w_t[:, :], in_=wot[:, :])
                wo_sb[0] = w_t

            def oproj_chunks(p):
                W = PW[p]
                gsb = gsb_all[p]
                for o in range(4):
                    for q0 in range(0, W, 512):
                        yield (p, o, q0, min(512, W - q0), gsb)

            def emit_oproj_chunk(chunk):
                p, o, q0, cw, gsb = chunk
                ps = pp.tile([128, cw], F32, name="psy", tag="pp")
                for t in range(HT):
                    nc.tensor.matmul(
                        ps[:, :],
                        wo_sb[0][:, 512 * t + 128 * o:
                                 512 * t + 128 * (o + 1)],
                        gsb[t][:, q0:q0 + cw],
                        start=(t == 0), stop=(t == HT - 1))
                y_t = ypool.tile([128, cw], F32, name="y", tag="y")
                nc.vector.tensor_scalar_add(y_t[:, :], ps[:, :],
                                            bo_sb[o][:, :])
                nc.sync.dma_start(
                    out=out[128 * o:128 * (o + 1),
                            PC0[p] + q0:PC0[p] + q0 + cw],
                    in_=y_t[:, :])

            def emit_oproj(p):
                for ch in oproj_chunks(p):
                    emit_oproj_chunk(ch)

            # phase 0 interleaved with Q projection; later Q chunks and
            # late V tiles drip into phases 0-2 inside attention chains
            for j in range(4):
                drain_fill()
                emit_qproj_chunk(j, 0)
                fill_q.append(qproj_gen(j, 1))
                fill_q.append(vproj_gen(4 + j))
                for qt in range(*PH[0]):
                    emit_attn_chain(j, qt)
            drain_fill()
            emit_wo_loads()
            emit_gather(0)
            for j in range(4):
                fill_q.append(qproj_gen(j, 2))
                if 8 + j < NT:
                    fill_q.append(vproj_gen(8 + j))
                for qt in range(*PH[1]):
                    emit_attn_chain(j, qt)
            drain_fill()
            emit_gather(1)
            for j in range(4):
                fill_q.append(qproj_gen(j, 3))
                if 12 + j < NT:
                    fill_q.append(vproj_gen(12 + j))
                for qt in range(*PH[2]):
                    emit_attn_chain(j, qt)
            drain_fill()
            emit_gather(2)
            for p in range(3, NPH):
                pending = list(oproj_chunks(p - 3))
                nchunk = len(pending)
                for j in range(4):
                    for qt in range(*PH[p]):
                        emit_attn_chain(j, qt)
                    for ch in pending[(nchunk * j + 3) // 4:
                                      (nchunk * (j + 1) + 3) // 4]:
                        emit_oproj_chunk(ch)
                emit_gather(p)
            emit_oproj(NPH - 3)
            emit_oproj(NPH - 2)
            emit_oproj(NPH - 1)

    nc.compile()
    return nc


def kernel(**inputs):
    global _COMPILED, LAST_EXEC_NS
    x = np.asarray(inputs["hidden_states"], dtype=np.float32)
    mask = np.asarray(inputs["attention_mask"], dtype=np.float32)
    pos = np.asarray(inputs["position_ids"])
    Wq = np.asarray(inputs["Wq"], dtype=np.float32)
    bq = np.asarray(inputs["bq"], dtype=np.float32)
    Wk = np.asarray(inputs["Wk"], dtype=np.float32)
    bk = np.asarray(inputs["bk"], dtype=np.float32)
    Wv = np.asarray(inputs["Wv"], dtype=np.float32)
    bv = np.asarray(inputs["bv"], dtype=np.float32)
    Wo = np.asarray(inputs["Wo"], dtype=np.float32)
    bo = np.asarray(inputs["bo"], dtype=np.float32)

    bf = ml_dtypes.bfloat16
    # rope tables (from the position_ids input)
    p = pos[0].astype(np.float32)
    inv = 1.0 / (10000.0 ** (np.arange(0, HD, 2, dtype=np.float32) / HD))
    fr = p[:, None] * inv[None, :]                       # (S, 32)
    emb = np.concatenate([fr, fr], axis=1)               # (S, 64)
    cosT = np.cos(emb).T.astype(np.float32)              # (64, S)
    sinT = np.sin(emb).T.astype(np.float32)
    # pre-shifted signed sin: multiplied at src rows, then shifted to dst
    ss_pre = np.concatenate([sinT[32:64], -sinT[0:32]], axis=0)  # (64, S)
    kcos = np.tile(cosT, (2, 1)).astype(bf)
    ksin = np.tile(ss_pre, (2, 1)).astype(bf)

    # all causal diagonal blocks are identical; ship one
    maskd = np.ascontiguousarray(mask[0, 0, 0:128, 0:128].T)

    # fold V bias into the output bias: softmax rows sum to 1, so the v-bias
    # contributes exactly Wo @ repeat_kv(bv) to every position.
    bv_full = np.concatenate(
        [bv[64 * (i // 4):64 * (i // 4) + 64] for i in range(NH)])
    bo_fold = (bo.astype(np.float64)
               + Wo.astype(np.float64) @ bv_full.astype(np.float64)
               ).astype(np.float32)

    def packw(wslice_t):
        # [H, cols] -> [128, HT*cols]: row-tiles side by side
        return np.ascontiguousarray(np.concatenate(
            [wslice_t[128 * t:128 * (t + 1)] for t in range(HT)], axis=1))

    in_maps = []
    for c in range(N_CORES):
        b, hg = c // 4, c % 4
        in_maps.append({
            "xt": np.ascontiguousarray(x[b].T).astype(bf),
            "wqt": packw(Wq[512 * hg:512 * (hg + 1), :].T * SCALE).astype(bf),
            "wkt": packw(Wk[128 * hg:128 * (hg + 1), :].T).astype(bf),
            "wvt": packw(Wv[128 * hg:128 * (hg + 1), :].T).astype(bf),
            "wot": packw(Wo[512 * hg:512 * (hg + 1), :].T).astype(bf),
            "bq": np.ascontiguousarray(bq[512 * hg:512 * (hg + 1)] * SCALE)[:, None],
            "bk": np.ascontiguousarray(bk[128 * hg:128 * (hg + 1)])[:, None],
            "bo": np.ascontiguousarray(bo_fold[512 * hg:512 * (hg + 1)])[:, None],
            "kcos": kcos, "ksin": ksin,
            "maskd": maskd,
        })

    if _COMPILED is None:
        _install_profile_shim()
        _COMPILED = _build()

    res = bass_utils.run_bass_kernel_spmd(
        _COMPILED, in_maps, core_ids=list(range(N_CORES)), trace=TRACE)
    LAST_EXEC_NS = res.exec_time_ns

    outb = []
    for b in range(B):
        yt = np.concatenate([res.results[4 * b + hg]["out"]
                             for hg in range(4)], axis=0)   # [2048 oc, 2048 q]
        outb.append(yt.T)
    return np.stack(outb).astype(np.float32)
